# revision 30
# baseline (speedup 1.0000x reference)
"""Block-diagonal linear (grouped GEMM) on 8 TRN2 NeuronCores.

out[b, g*512+n] = sum_k x[b, g*512+k] * blocks[g, k, n]

Sharding: group-parallel — core g computes block g's GEMM. The host hands
each core xT = x[:, g*512:(g+1)*512].T ([512, 8192], feature-major) in
bf16 and receives outT ([512, 8192]) in bf16; transposes and dtype
conversion happen on the host so the device needs no PE transposes and
every DMA stream reads/writes long contiguous runs per partition.

bf16 halves HBM traffic vs fp32 (the fp32 version was DMA-bound at
~34.6MB/core ~= 96us; bf16 is ~17.3MB ~= 48us) while the PE runs bf16 at
the same 1 cycle/row as f32r, so the kernel is compute-bound at ~55-66us
(PE at 2.4 or 2.0 GHz depending on the chip's power state). Accumulation
stays fp32 in PSUM; end-to-end max rel err vs the fp32 reference is
~4e-3 (gate 2e-2).

Per-core kernel: out.T = W.T @ x.T as PSUM accumulation groups:
psum[n-tile 128, m 512] += W[k-tile, n-tile].T @ xT[k-tile, m-chunk].
A run of dependency-free dummy matmuls at the start keeps the PE busy
while the first DMAs land, so the HAM clock-gate un-throttles (1.2 ->
full GHz) right as the real stream begins.
"""
import numpy as np
import ml_dtypes

import concourse.bacc as bacc
import concourse.tile as tile
from concourse import mybir
from concourse.bass_utils import run_bass_kernel_spmd

TOKENS = 8192
G = 8
M = 512  # per-block in-features
N = 512  # per-block out-features
P = 128
KT = M // P  # 4 contraction tiles
NT = N // P  # 4 output feature tiles
SUB = 512    # tokens per PSUM group (one fp32 PSUM bank)
F32 = mybir.dt.float32
BF16 = mybir.dt.bfloat16
NPBF16 = ml_dtypes.bfloat16

# token-chunk schedule: small head for pipeline ramp, 2048 steady, tapered
# tail so the last casts/DMAs drain right behind the last matmul
CHUNKS = [512, 512, 1024, 1024, 2048, 2048, 512, 384, 128]
assert sum(CHUNKS) == TOKENS
CMAX = max(CHUNKS)
N_WARM = 40  # dummy matmuls that keep the PE busy while the first DMAs land

_CACHE: dict = {}


def _body(tc, nc, xT, w, outT):
    with (
        tc.tile_pool(name="wp", bufs=1) as wp,
        tc.tile_pool(name="xin", bufs=12) as xin,
        tc.tile_pool(name="outp", bufs=2) as outp,
        tc.tile_pool(name="pso", bufs=8, space="PSUM") as pso,
    ):
        # weights [512, 512] bf16 -> [128, kt, 512]
        w_r = wp.tile([P, KT, N], BF16, tag="wr")
        w_v = w.rearrange("(j p) n -> j p n", p=P)

        # HAM warm-up: the PE only reaches full clock after ~3.4us of
        # sustained busy.  Burn that window on dependency-free dummy matmuls
        # over zeroed SBUF into a scratch PSUM bank (never read) while the
        # first real DMAs are still in flight, so the real matmul stream
        # starts at full rate.
        warm_x = xin.tile([P, CMAX], BF16, tag="x")
        warm_ps = pso.tile([P, SUB], F32, tag="pso")
        nc.vector.memset(warm_x[:, :2 * P], 0)
        for _ in range(N_WARM):
            nc.tensor.matmul(
                warm_ps[:, :P], warm_x[:, :P], warm_x[:, P:2 * P],
                start=True, stop=True,
            )

        m0 = 0
        for ci, c in enumerate(CHUNKS):
            # load the 4 k-tiles of this token chunk, striped across
            # the two HWDGE rings (sync=SP and scalar=ACT)
            xs = []
            for j in range(KT):
                x_t = xin.tile([P, CMAX], BF16, tag="x")
                eng = nc.sync if j % 2 == 0 else nc.scalar
                eng.dma_start(x_t[:, :c], xT[j * P:(j + 1) * P, m0:m0 + c])
                xs.append(x_t)
            if ci == 0:
                # W rides both rings right behind the first chunk
                for j in range(KT):
                    eng = nc.sync if j % 2 == 0 else nc.scalar
                    eng.dma_start(w_r[:, j, :], w_v[j])

            ots = [outp.tile([P, CMAX], BF16, tag=f"o{nt}", name=f"ot{nt}") for nt in range(NT)]
            for s0 in range(0, c, SUB):
                sw = min(SUB, c - s0)
                for nt in range(NT):
                    ps_o = pso.tile([P, SUB], F32, tag="pso")
                    for j in range(KT):
                        nc.tensor.matmul(
                            ps_o[:, :sw],
                            w_r[:, j, nt * P:(nt + 1) * P],
                            xs[j][:, s0:s0 + sw],
                            start=(j == 0),
                            stop=(j == KT - 1),
                        )
                    nc.vector.tensor_copy(ots[nt][:, s0:s0 + sw], ps_o[:, :sw])
            # flush the chunk: one DMA per n-tile on the SWDGE ring; the last
            # chunks ride the HWDGE rings (input traffic is done by then)
            for nt in range(NT):
                if ci >= len(CHUNKS) - 3:
                    eng = nc.sync if nt % 2 == 0 else nc.scalar
                else:
                    eng = nc.gpsimd
                eng.dma_start(outT[nt * P:(nt + 1) * P, m0:m0 + c], ots[nt][:, :c])
            m0 += c


def _build():
    nc = bacc.Bacc("TRN2", target_bir_lowering=False, debug=False, num_devices=G)
    xT = nc.dram_tensor("xT", [M, TOKENS], BF16, kind="ExternalInput").ap()
    w = nc.dram_tensor("w", [M, N], BF16, kind="ExternalInput").ap()
    outT = nc.dram_tensor("outT", [N, TOKENS], BF16, kind="ExternalOutput").ap()
    with tile.TileContext(nc) as tc:
        _body(tc, nc, xT, w, outT)
    nc.compile()
    return nc


def _run(in_maps, **kwargs):
    if "nc" not in _CACHE:
        _CACHE["nc"] = _build()
    return run_bass_kernel_spmd(_CACHE["nc"], in_maps, list(range(G)), **kwargs)


def _in_maps(x, blocks):
    return [
        {
            "xT": np.ascontiguousarray(x[:, g * M:(g + 1) * M].T).astype(NPBF16),
            "w": np.ascontiguousarray(blocks[g]).astype(NPBF16),
        }
        for g in range(G)
    ]


def kernel(x, blocks):
    x = np.asarray(x)
    blocks = np.asarray(blocks)
    res = _run(_in_maps(x, blocks))
    return np.concatenate(
        [res.results[g]["outT"].T.astype(np.float32) for g in range(G)], axis=1
    )
